# revision 1
# baseline (speedup 1.0000x reference)
"""Chamfer-distance kernel for TRN2 (8 NeuronCores, SPMD).

Math: the reference weights w are nonzero ONLY for points with
time_indice == 1 (m of N points).  So of the NxN distance matrix we only
need row-mins for the m selected rows (dist1) and col-mins for the m
selected columns (dist2) -- each an (m x N) problem, min over N.

Each (m x N) pass is computed as a K=4 matmul:
    C[i, j] = sq[j] - 2 * dot(sel_i, pts_j)
with lhsT rows 0..2 = -2*sel coords, row 3 = ones, and rhs rows 0..2 =
pts coords, row 3 = |pts|^2.  Row-min over j then happens on the Vector
engine (tensor_reduce min straight out of PSUM).  The per-row constant
sq[i] of the selected point is added on the host after the global min.

Sharding: the N search points are split 2048-per-core across 8 cores
(same lhsT everywhere); each core returns per-row partial mins, the host
takes the elementwise min across cores and does the tiny O(m) tail.
"""

import numpy as np

import concourse.bass as bass
import concourse.mybir as mybir
import concourse.tile as tile
from concourse import bacc
from concourse.bass_utils import run_bass_kernel_spmd

N_CORES = 8
N_POINTS = 16384
NSHARD = N_POINTS // N_CORES  # 2048 search points per core
FREE = 512                    # matmul moving free dim (one PSUM bank of fp32)

_CACHE = {}


def _build(n_rt):
    """Build + compile the SPMD Bass program for n_rt row-tiles of 128."""
    f32 = mybir.dt.float32
    mpad = n_rt * 128
    ncc = NSHARD // FREE

    nc = bacc.Bacc("TRN2", target_bir_lowering=False, debug=False,
                   num_devices=N_CORES)
    lhsA = nc.dram_tensor("lhsA", [4, mpad], f32, kind="ExternalInput").ap()
    rhsA = nc.dram_tensor("rhsA", [4, NSHARD], f32, kind="ExternalInput").ap()
    lhsB = nc.dram_tensor("lhsB", [4, mpad], f32, kind="ExternalInput").ap()
    rhsB = nc.dram_tensor("rhsB", [4, NSHARD], f32, kind="ExternalInput").ap()
    outA = nc.dram_tensor("outA", [128, n_rt], f32, kind="ExternalOutput").ap()
    outB = nc.dram_tensor("outB", [128, n_rt], f32, kind="ExternalOutput").ap()

    with tile.TileContext(nc) as tc:
        with (
            tc.tile_pool(name="inp", bufs=1) as inp,
            tc.tile_pool(name="res", bufs=1) as res,
            tc.tile_pool(name="ps", bufs=2, space="PSUM") as ps,
        ):
            lA = inp.tile([4, mpad], f32, tag="lA")
            nc.sync.dma_start(out=lA[:], in_=lhsA)
            rA = inp.tile([4, NSHARD], f32, tag="rA")
            nc.sync.dma_start(out=rA[:], in_=rhsA)
            lB = inp.tile([4, mpad], f32, tag="lB")
            nc.sync.dma_start(out=lB[:], in_=lhsB)
            rB = inp.tile([4, NSHARD], f32, tag="rB")
            nc.sync.dma_start(out=rB[:], in_=rhsB)

            mA = res.tile([128, n_rt], f32, tag="mA")
            mB = res.tile([128, n_rt], f32, tag="mB")

            for lhs, rhs, mins in ((lA, rA, mA), (lB, rB, mB)):
                for rt in range(n_rt):
                    pt = ps.tile([128, NSHARD], f32, tag="ps")
                    for cc in range(ncc):
                        nc.tensor.matmul(
                            pt[:, bass.ts(cc, FREE)],
                            lhs[:, bass.ts(rt, 128)],
                            rhs[:, bass.ts(cc, FREE)],
                            start=True, stop=True,
                        )
                    nc.vector.tensor_reduce(
                        mins[:, rt:rt + 1], pt[:, :],
                        axis=mybir.AxisListType.X, op=mybir.AluOpType.min,
                    )

            nc.sync.dma_start(out=outA, in_=mA[:])
            nc.sync.dma_start(out=outB, in_=mB[:])

    nc.compile()
    return nc


def _get_program(n_rt):
    if n_rt not in _CACHE:
        _CACHE[n_rt] = _build(n_rt)
    return _CACHE[n_rt]


def _transform(points, poses, idx):
    P = poses[idx]                                   # [N,4,4]
    R, t = P[:, :3, :3], P[:, :3, 3]
    return np.einsum('nij,nj->ni', R, points) + t    # [N,3]


def kernel(points, time_indice, est_poses, gt_poses):
    points = np.asarray(points, dtype=np.float32)
    ti = np.asarray(time_indice)
    est_poses = np.asarray(est_poses, dtype=np.float32)
    gt_poses = np.asarray(gt_poses, dtype=np.float32)

    est = _transform(points, est_poses, ti)          # [N,3]
    gt = _transform(points, gt_poses, ti)            # [N,3]
    est_sq = np.sum(est * est, axis=1)               # [N]
    gt_sq = np.sum(gt * gt, axis=1)                  # [N]

    sel = np.flatnonzero(ti == 1)
    m = sel.size
    denom = np.float32(m) + np.float32(1e-7)
    if m == 0:
        return np.float32(0.0), np.float32(0.0)

    l2 = np.float32(
        np.linalg.norm((est[sel] - gt[sel]).astype(np.float64), axis=1).sum()
        / denom)

    n_rt = -(-m // 128)
    mpad = n_rt * 128
    pad = np.concatenate([sel, np.repeat(sel[:1], mpad - m)])

    def lhs_for(sel_pts):
        out = np.empty((4, mpad), np.float32)
        out[:3] = (-2.0 * sel_pts[pad]).T
        out[3] = 1.0
        return out

    def rhs_for(pts, sq, c):
        s = slice(c * NSHARD, (c + 1) * NSHARD)
        out = np.empty((4, NSHARD), np.float32)
        out[:3] = pts[s].T
        out[3] = sq[s]
        return out

    lhsA = lhs_for(gt)    # dist1: selected gt rows vs all est points
    lhsB = lhs_for(est)   # dist2: selected est rows vs all gt points
    in_maps = [
        {
            "lhsA": lhsA,
            "rhsA": rhs_for(est, est_sq, c),
            "lhsB": lhsB,
            "rhsB": rhs_for(gt, gt_sq, c),
        }
        for c in range(N_CORES)
    ]

    nc = _get_program(n_rt)
    results = run_bass_kernel_spmd(nc, in_maps, list(range(N_CORES))).results

    # [128, n_rt] per core -> global min across cores -> flatten row-tiles
    partA = np.min([r["outA"] for r in results], axis=0).T.ravel()[:m]
    partB = np.min([r["outB"] for r in results], axis=0).T.ravel()[:m]
    dist1 = partA.astype(np.float64) + gt_sq[sel]
    dist2 = partB.astype(np.float64) + est_sq[sel]
    chamfer = np.float32(0.5 * (dist1.sum() + dist2.sum()) / denom)
    return chamfer, l2


# revision 3
# speedup vs baseline: 2.5698x; 2.5698x over previous
"""Chamfer-distance kernel for TRN2 (8 NeuronCores, SPMD).

Math: the reference weights w are nonzero ONLY for points with
time_indice == 1 (m of N points).  So of the NxN distance matrix we only
need row-mins for the m selected rows (dist1) and col-mins for the m
selected columns (dist2) -- each an (m x N) problem, min over N.

Each (m x N) pass is computed as a K=4 matmul:
    C[i, j] = sq[j] - 2 * dot(sel_i, pts_j)
with lhsT rows 0..2 = -2*sel coords, row 3 = ones, and rhs rows 0..2 =
pts coords, row 3 = |pts|^2.  Row-min over j then happens on the Vector
engine (tensor_reduce min straight out of PSUM).  The per-row constant
sq[i] of the selected point is added on the host after the global min.

Sharding: the N search points are split 2048-per-core across 8 cores
(same lhsT everywhere); each core returns per-row partial mins, the host
takes the elementwise min across cores and does the tiny O(m) tail.
"""

import numpy as np

import concourse.bass as bass
import concourse.mybir as mybir
import concourse.tile as tile
from concourse import bacc
from concourse.bass_utils import run_bass_kernel_spmd

N_CORES = 8
N_POINTS = 16384
NSHARD = N_POINTS // N_CORES  # 2048 search points per core
FREE = 512                    # matmul moving free dim (one PSUM bank of fp32)

_CACHE = {}

# dtype used for the matmul operands: float32r streams 1 col/cycle on the
# PE (vs 4 for float32's LOW_HIGH dual pass) at reduced internal precision.
MM_DT = "float32r"


def _build(n_rt):
    """Build + compile the SPMD Bass program for n_rt row-tiles of 128."""
    f32 = mybir.dt.float32
    mdt = getattr(mybir.dt, MM_DT)
    mpad = n_rt * 128
    ncc = NSHARD // FREE

    nc = bacc.Bacc("TRN2", target_bir_lowering=False, debug=False,
                   num_devices=N_CORES)
    lhsA = nc.dram_tensor("lhsA", [4, mpad], mdt, kind="ExternalInput").ap()
    rhsA = nc.dram_tensor("rhsA", [4, NSHARD], mdt, kind="ExternalInput").ap()
    lhsB = nc.dram_tensor("lhsB", [4, mpad], mdt, kind="ExternalInput").ap()
    rhsB = nc.dram_tensor("rhsB", [4, NSHARD], mdt, kind="ExternalInput").ap()
    outA = nc.dram_tensor("outA", [128, n_rt], f32, kind="ExternalOutput").ap()
    outB = nc.dram_tensor("outB", [128, n_rt], f32, kind="ExternalOutput").ap()

    with tile.TileContext(nc) as tc:
        with (
            tc.tile_pool(name="inp", bufs=1) as inp,
            tc.tile_pool(name="res", bufs=1) as res,
            tc.tile_pool(name="ps", bufs=2, space="PSUM") as ps,
        ):
            lA = inp.tile([4, mpad], mdt, tag="lA")
            nc.sync.dma_start(out=lA[:], in_=lhsA)
            rA = inp.tile([4, NSHARD], mdt, tag="rA")
            nc.sync.dma_start(out=rA[:], in_=rhsA)
            lB = inp.tile([4, mpad], mdt, tag="lB")
            nc.sync.dma_start(out=lB[:], in_=lhsB)
            rB = inp.tile([4, NSHARD], mdt, tag="rB")
            nc.sync.dma_start(out=rB[:], in_=rhsB)

            mA = res.tile([128, n_rt], f32, tag="mA")
            mB = res.tile([128, n_rt], f32, tag="mB")

            for lhs, rhs, mins in ((lA, rA, mA), (lB, rB, mB)):
                for rt in range(n_rt):
                    pt = ps.tile([128, NSHARD], f32, tag="ps")
                    for cc in range(ncc):
                        nc.tensor.matmul(
                            pt[:, bass.ts(cc, FREE)],
                            lhs[:, bass.ts(rt, 128)],
                            rhs[:, bass.ts(cc, FREE)],
                            start=True, stop=True,
                        )
                    nc.vector.tensor_reduce(
                        mins[:, rt:rt + 1], pt[:, :],
                        axis=mybir.AxisListType.X, op=mybir.AluOpType.min,
                    )

            nc.sync.dma_start(out=outA, in_=mA[:])
            nc.sync.dma_start(out=outB, in_=mB[:])

    nc.compile()
    return nc


def _get_program(n_rt):
    if n_rt not in _CACHE:
        _CACHE[n_rt] = _build(n_rt)
    return _CACHE[n_rt]


def _transform(points, poses, idx):
    P = poses[idx]                                   # [N,4,4]
    R, t = P[:, :3, :3], P[:, :3, 3]
    return np.einsum('nij,nj->ni', R, points) + t    # [N,3]


def kernel(points, time_indice, est_poses, gt_poses):
    points = np.asarray(points, dtype=np.float32)
    ti = np.asarray(time_indice)
    est_poses = np.asarray(est_poses, dtype=np.float32)
    gt_poses = np.asarray(gt_poses, dtype=np.float32)

    est = _transform(points, est_poses, ti)          # [N,3]
    gt = _transform(points, gt_poses, ti)            # [N,3]
    est_sq = np.sum(est * est, axis=1)               # [N]
    gt_sq = np.sum(gt * gt, axis=1)                  # [N]

    sel = np.flatnonzero(ti == 1)
    m = sel.size
    denom = np.float32(m) + np.float32(1e-7)
    if m == 0:
        return np.float32(0.0), np.float32(0.0)

    l2 = np.float32(
        np.linalg.norm((est[sel] - gt[sel]).astype(np.float64), axis=1).sum()
        / denom)

    n_rt = -(-m // 128)
    mpad = n_rt * 128
    pad = np.concatenate([sel, np.repeat(sel[:1], mpad - m)])

    def lhs_for(sel_pts):
        out = np.empty((4, mpad), np.float32)
        out[:3] = (-2.0 * sel_pts[pad]).T
        out[3] = 1.0
        return out

    def rhs_for(pts, sq, c):
        s = slice(c * NSHARD, (c + 1) * NSHARD)
        out = np.empty((4, NSHARD), np.float32)
        out[:3] = pts[s].T
        out[3] = sq[s]
        return out

    lhsA = lhs_for(gt)    # dist1: selected gt rows vs all est points
    lhsB = lhs_for(est)   # dist2: selected est rows vs all gt points
    in_maps = [
        {
            "lhsA": lhsA,
            "rhsA": rhs_for(est, est_sq, c),
            "lhsB": lhsB,
            "rhsB": rhs_for(gt, gt_sq, c),
        }
        for c in range(N_CORES)
    ]

    nc = _get_program(n_rt)
    results = run_bass_kernel_spmd(nc, in_maps, list(range(N_CORES))).results

    # [128, n_rt] per core -> global min across cores -> flatten row-tiles
    partA = np.min([r["outA"] for r in results], axis=0).T.ravel()[:m]
    partB = np.min([r["outB"] for r in results], axis=0).T.ravel()[:m]
    dist1 = partA.astype(np.float64) + gt_sq[sel]
    dist2 = partB.astype(np.float64) + est_sq[sel]
    chamfer = np.float32(0.5 * (dist1.sum() + dist2.sum()) / denom)
    return chamfer, l2
